# revision 17
# baseline (speedup 1.0000x reference)
"""DFSMN layer Trainium2 kernel (8-core SPMD, batch-parallel).

Math: per batch b,
  h = x @ W^T + b_lin                      [L, H]
  out_pre[t] = h[t] + mem[t] + fut[t]  ==  (M @ h)[t]
    with M [L, L] banded: identity + past taps (50) + future taps (5),
    taps are scalars per lag: wm = mem_w.sum(-1), wf = la_w.sum(-1).
  out = LayerNorm_H(out_pre) * gamma + beta

On device (per core = one batch):
  g = x @ W^T        (bf16 TensorE, fp32 PSUM; x shipped pre-transposed)
  pre = M @ g + s (x) b_lin   (block-banded TensorE matmuls; s = M row sums
                               folds the bias through the taps: M @ (1 b^T) = s b^T)
  out = (pre - mean) * rsqrt(var + eps)  (DVE bn_stats/bn_aggr)
"""
import numpy as np
import ml_dtypes

MEM, LA, EPS = 50, 5, 1e-5
B, L, D, H = 8, 2048, 1024, 2048
NCORES = 8
PT = 128              # time tile (partition dim)
TB = L // PT          # 16 time tiles
DC = D // PT          # 8 contract chunks
HN = 512              # matmul moving free dim
HC = H // HN          # 4 H chunks

# Static band-block pattern: (tb, sb) pairs, sb in {tb-1, tb, tb+1} clipped.
BLOCK_LIST = [(tb, sb) for tb in range(TB)
              for sb in (tb - 1, tb, tb + 1) if 0 <= sb < TB]
NBLK = len(BLOCK_LIST)
BLOCKS_BY_TB = {tb: [(k, sb) for k, (tb2, sb) in enumerate(BLOCK_LIST) if tb2 == tb]
                for tb in range(TB)}

_cached = {}
last_exec_time_ns = None


def _band_matrix(wm, wf):
    """M [L, L] fp32: out_pre = M @ h. Returns (M, row_sums)."""
    M = np.zeros((L, L), np.float32)
    idx = np.arange(L)
    M[idx, idx] = 1.0
    for t in range(L):
        if t < MEM:
            M[t, :t] += wm[:t]
        else:
            M[t, t - MEM:t] += wm
        hi = min(t + LA, L - 1)
        if hi >= t + 1:
            M[t, t + 1:hi + 1] += wf[:hi - t]
    return M, M.sum(axis=1)


def _build_nc(reps=1, loop_k=None):
    from concourse import bacc
    import concourse.mybir as mybir
    import concourse.tile as tile

    dt = mybir.dt.bfloat16
    f32 = mybir.dt.float32
    sub = mybir.AluOpType.subtract
    mult = mybir.AluOpType.mult

    nc = bacc.Bacc(None, target_bir_lowering=False)
    # x shipped transposed and t-tile-major: [TB, D, PT] so tile i's lhsT
    # slices are one small contiguous region per (i, dc).
    xtT = nc.declare_dram_parameter("xtT", [TB, D, PT], dt, isOutput=False)
    wT = nc.declare_dram_parameter("wT", [D, H], dt, isOutput=False)
    mT = nc.declare_dram_parameter("mT", [PT, NBLK, PT], dt, isOutput=False)
    sv = nc.declare_dram_parameter("sv", [1, L], dt, isOutput=False)
    bv = nc.declare_dram_parameter("bv", [1, H], dt, isOutput=False)
    out = nc.declare_dram_parameter("out", [L, H], f32, isOutput=True)

    with tile.TileContext(nc) as tc:
        with tc.tile_pool(name="const", bufs=1) as const, \
             tc.tile_pool(name="gpool", bufs=4) as gpool, \
             tc.tile_pool(name="opool", bufs=2) as opool, \
             tc.tile_pool(name="ln", bufs=2) as ln, \
             tc.tile_pool(name="psg", bufs=1, space="PSUM") as psg, \
             tc.tile_pool(name="psp", bufs=1, space="PSUM") as psp:

            wt_tiles = []
            for dc in range(DC):
                w = const.tile([PT, H], dt, tag=f"wt{dc}")
                nc.sync.dma_start(out=w, in_=wT[dc * PT:(dc + 1) * PT, :])
                wt_tiles.append(w)
            # x tiles: [PT(dc partition), TB, PT(t)] — per (tile, dc) slice is
            # xt_tiles[dc][:, i, :]; loaded t-tile-major so tile 0 is ready
            # after its first 256KB instead of the whole 4MB.
            xt_tiles = []
            for dc in range(DC):
                t = const.tile([PT, TB, PT], dt, tag=f"xt{dc}")
                xt_tiles.append(t)
            for i in range(TB):
                for dc in range(DC):
                    nc.sync.dma_start(
                        out=xt_tiles[dc][:, i, :],
                        in_=xtT[i, dc * PT:(dc + 1) * PT, :])
            mt_t = const.tile([PT, NBLK, PT], dt, tag="mt")
            nc.sync.dma_start(out=mt_t, in_=mT[:, :, :])
            sv_t = const.tile([1, L], dt, tag="sv")
            nc.sync.dma_start(out=sv_t, in_=sv[:, :])
            bv_t = const.tile([1, H], dt, tag="bv")
            nc.sync.dma_start(out=bv_t, in_=bv[:, :])
            eps_t = const.tile([PT, 1], f32, tag="eps")
            nc.vector.memset(eps_t, EPS)

            if loop_k is not None:
                with tc.For_i(0, loop_k, 1):
                    _emit_body(nc, mybir, xt_tiles, wt_tiles, mt_t, sv_t, bv_t,
                               eps_t, gpool, opool, ln, psg, psp, out, sub, mult)
            else:
                for _rep in range(reps):
                    _emit_body(nc, mybir, xt_tiles, wt_tiles, mt_t, sv_t, bv_t,
                               eps_t, gpool, opool, ln, psg, psp, out, sub, mult)
    nc.finalize()
    return nc


def _emit_body(nc, mybir, xt_tiles, wt_tiles, mt_t, sv_t, bv_t, eps_t,
               gpool, opool, ln, psg, psp, out, sub, mult):
    dt = mybir.dt.bfloat16
    f32 = mybir.dt.float32
    if True:
        if True:
            g_tiles = [None] * TB
            for i in range(TB + 1):
                if i < TB:
                    # g[i] = x-tile @ W^T: dc-outer so each weight (xt slice)
                    # loads once per tile; 4 psum banks accumulate in parallel.
                    pg = psg.tile([PT, H], f32, tag="pg")
                    for dc in range(DC):
                        for hc in range(HC):
                            nc.tensor.matmul(
                                pg[:, hc * HN:(hc + 1) * HN],
                                xt_tiles[dc][:, i, :],
                                wt_tiles[dc][:, hc * HN:(hc + 1) * HN],
                                start=(dc == 0), stop=(dc == DC - 1))
                    g = gpool.tile([PT, H], dt, tag="g")
                    g_tiles[i] = g
                    nc.scalar.copy(out=g, in_=pg)  # bf16 copy for the band stage
                if i >= 1:
                    # band + bias for tile j (needs g[j-1], g[j], g[j+1])
                    j = i - 1
                    pre = psp.tile([PT, H], f32, tag="pre")
                    blist = BLOCKS_BY_TB[j]
                    for hc in range(HC):
                        hs = slice(hc * HN, (hc + 1) * HN)
                        for bi, (k, sb) in enumerate(blist):
                            nc.tensor.matmul(
                                pre[:, hs], mt_t[:, k, :], g_tiles[sb][:, hs],
                                start=(bi == 0), stop=False)
                        nc.tensor.matmul(
                            pre[:, hs], sv_t[:, j * PT:(j + 1) * PT], bv_t[:, hs],
                            start=False, stop=True)
                    # Evacuate PSUM early (ScalarE sits close to PSUM); LN
                    # then runs from SBUF where tensor_scalar gets 2x mode.
                    pre_sb = opool.tile([PT, H], f32, tag="presb")
                    nc.scalar.copy(out=pre_sb, in_=pre)
                    # LayerNorm over H
                    stats = ln.tile([PT, HC, 6], f32, tag="stats")
                    for hc in range(HC):
                        nc.vector.bn_stats(out=stats[:, hc, :],
                                           in_=pre_sb[:, hc * HN:(hc + 1) * HN])
                    mv = ln.tile([PT, 2], f32, tag="mv")
                    nc.vector.bn_aggr(out=mv, in_=stats)
                    rstd = ln.tile([PT, 1], f32, tag="rstd")
                    nc.scalar.activation(
                        out=rstd, in_=mv[:, 1:2],
                        func=mybir.ActivationFunctionType.Sqrt,
                        bias=eps_t, scale=1.0)
                    nc.vector.reciprocal(out=rstd, in_=rstd)
                    o = opool.tile([PT, H], f32, tag="o")
                    nc.vector.tensor_scalar(
                        out=o, in0=pre_sb, scalar1=mv[:, 0:1], scalar2=rstd,
                        op0=sub, op1=mult)
                    nc.sync.dma_start(out=out[j * PT:(j + 1) * PT, :], in_=o)


def _get_runner(reps=1):
    """Compile once; return (run_fn, in_names, out_names).

    run_fn takes a list of global (concatenated-over-cores) jax/np arrays in
    in_names order followed by zero output buffers, returns global outputs.
    Mirrors concourse.bass2jax.run_bass_via_pjrt's multi-core branch, but
    keeps the jitted callable so repeated invocations don't rebuild/retrace.
    """
    key = ("runner", reps)
    if key in _cached:
        return _cached[key]

    import jax
    from jax.experimental.shard_map import shard_map
    from jax.sharding import Mesh, PartitionSpec
    import concourse.mybir as mybir
    from concourse import bass2jax

    if isinstance(reps, tuple):  # ("loop", K): hardware For_i timing variant
        nc = _build_nc(loop_k=reps[1])
    else:
        nc = _build_nc(reps)
    bass2jax.install_neuronx_cc_hook()

    partition_name = nc.partition_id_tensor.name if nc.partition_id_tensor else None
    in_names, out_names, out_avals, zero_outs = [], [], [], []
    for alloc in nc.m.functions[0].allocations:
        if not isinstance(alloc, mybir.MemoryLocationSet):
            continue
        name = alloc.memorylocations[0].name
        if alloc.kind == "ExternalInput":
            if name != partition_name:
                in_names.append(name)
        elif alloc.kind == "ExternalOutput":
            out_names.append(name)
            shape = tuple(alloc.tensor_shape)
            dtype = mybir.dt.np(alloc.dtype)
            out_avals.append(jax.core.ShapedArray(shape, dtype))
            zero_outs.append(np.zeros(shape, dtype))
    n_params = len(in_names)
    all_names = in_names + out_names
    if partition_name is not None:
        all_names.append(partition_name)

    def _body(*args):
        operands = list(args)
        if partition_name is not None:
            operands.append(bass2jax.partition_id_tensor())
        outs = bass2jax._bass_exec_p.bind(
            *operands,
            out_avals=tuple(out_avals),
            in_names=tuple(all_names),
            out_names=tuple(out_names),
            lowering_input_output_aliases=(),
            sim_require_finite=True,
            sim_require_nnan=True,
            nc=nc,
        )
        return tuple(outs)

    devices = jax.devices()[:NCORES]
    assert len(devices) == NCORES, f"need {NCORES} devices, have {len(jax.devices())}"
    mesh = Mesh(np.asarray(devices), ("core",))
    n_outs = len(out_names)
    fn = jax.jit(shard_map(
        _body, mesh=mesh,
        in_specs=(PartitionSpec("core"),) * (n_params + n_outs),
        out_specs=(PartitionSpec("core"),) * n_outs,
        check_rep=False))

    _cached[key] = (fn, in_names, out_names, zero_outs, mesh)
    return _cached[key]


def _prepare_in_arrays(x, W_lin, b_lin, wm, wf):
    """Host prep: per-core inputs concatenated over the core axis (axis 0)."""
    bf16 = ml_dtypes.bfloat16
    M, s = _band_matrix(wm, wf)
    mt_host = np.empty((PT, NBLK, PT), np.float32)
    for k, (tb, sb) in enumerate(BLOCK_LIST):
        mt_host[:, k, :] = M[tb * PT:(tb + 1) * PT, sb * PT:(sb + 1) * PT].T
    per_core = {
        "wT": np.ascontiguousarray(W_lin.T).astype(bf16),
        "mT": mt_host.astype(bf16),
        "sv": s.reshape(1, L).astype(bf16),
        "bv": b_lin.reshape(1, H).astype(bf16),
    }
    arrays = {}
    # x: per-core transposed, t-tile-major: [B, TB, D, PT]
    xt = np.ascontiguousarray(
        x.reshape(B, TB, PT, D).transpose(0, 1, 3, 2)).astype(bf16)
    arrays["xtT"] = xt.reshape(B * TB, D, PT)
    for name, arr in per_core.items():
        arrays[name] = np.concatenate([arr] * NCORES, axis=0)
    return arrays


def _run(arrays):
    fn, in_names, out_names, zero_outs, _ = _get_runner()
    global_zero = [np.concatenate([z] * NCORES, axis=0) for z in zero_outs]
    args = [arrays[n] for n in in_names] + global_zero
    outs = fn(*args)
    return {n: np.asarray(o) for n, o in zip(out_names, outs)}


def kernel(x, W_lin, b_lin, mem_w, la_w, gamma, beta):
    x = np.asarray(x, np.float32)
    W_lin = np.asarray(W_lin, np.float32)
    b_lin = np.asarray(b_lin, np.float32)
    wm = np.asarray(mem_w, np.float32).sum(axis=-1, dtype=np.float32)
    wf = np.asarray(la_w, np.float32).sum(axis=-1, dtype=np.float32)
    gamma = np.asarray(gamma, np.float32)
    beta = np.asarray(beta, np.float32)

    arrays = _prepare_in_arrays(x, W_lin, b_lin, wm, wf)
    outs = _run(arrays)
    out = outs["out"].reshape(NCORES, L, H)

    # gamma/beta affine (trivial for the spec's ones/zeros fills; exact in general)
    if not np.all(gamma == 1.0):
        out = out * gamma[None, None, :]
    if not np.all(beta == 0.0):
        out = out + beta[None, None, :]
    return np.ascontiguousarray(out.astype(np.float32))
